# revision 16
# baseline (speedup 1.0000x reference)
"""Trainium2 Bass kernel for a Neural ODE (RK4, 100 steps, 2-layer tanh MLP field).

Strategy: data-parallel over batch (4096 -> 8 cores x 512). Per core the batch
is split into 2 interleaved streams of 256 columns so the serial RK4 dependency
chain of one stream overlaps the other stream's engine work.

Layout: state kept transposed, xT[p, j, b] = x[b, j*128+p] (D=256 = 2 halves of
128 partitions). Matmuls run in bf16 (weights + activations + stage points)
with fp32 PSUM accumulation; the integration state x stays fp32 and is updated
with fused vector ops, so no precision is lost in the accumulated trajectory.

RK4 via scaled weights (h baked into W2 copies on host):
  q1 = (h/2) k1', q2 = (h/2) k2', q3 = (h/3) k3', q4 = (h/6) k4'
  a2 = x + q1, a3 = x + q2, a4 = x + 3*q3   (biases folded into tanh bias)
  xn = ((a2 + 2 a3)/3 + q3) + q4 + h*b2
"""
import numpy as np

import concourse.bass as bass
import concourse.mybir as mybir
from concourse import bacc
from concourse.tile import TileContext
from concourse.bass_utils import run_bass_kernel_spmd

DT = 0.01
N_STEPS = 100
B, D, H = 4096, 256, 256
NCORES = 8
BC = B // NCORES        # 512 batch per core
STREAM_SIZES = (256, 256)   # batch per stream (sums to BC); fp32r matmul
STREAM_OFFS = (0, 256)      # needs moving-dim >= 256 for full PE rate
N_STREAMS = len(STREAM_SIZES)
P = 128
ND2 = 2                 # D/H halves

F32 = mybir.dt.float32
F32R = mybir.dt.float32r
BF16 = mybir.dt.bfloat16
ADD = mybir.AluOpType.add
MULT = mybir.AluOpType.mult
TANH = mybir.ActivationFunctionType.Tanh
COPY = mybir.ActivationFunctionType.Copy


def build_kernel(n_steps=N_STEPS, with_bias=False, split_tanh=False):
    from contextlib import contextmanager
    nc = bacc.Bacc(None, target_bir_lowering=False)

    x0_d = nc.declare_dram_parameter("x0t", [ND2, P, BC], F32, isOutput=False)
    w1_d = nc.declare_dram_parameter("w1t", [P, ND2, ND2, P], F32R, isOutput=False)
    w2h_d = nc.declare_dram_parameter("w2h", [P, ND2, ND2, P], BF16, isOutput=False)
    w2f_d = nc.declare_dram_parameter("w2f", [P, ND2, ND2, P], BF16, isOutput=False)
    w2s_d = nc.declare_dram_parameter("w2s", [P, ND2, ND2, P], BF16, isOutput=False)
    # tanh biases per eval (4) per half (2): bias[p, e, j]
    bias_d = nc.declare_dram_parameter("biases", [P, 4, ND2], F32, isOutput=False)
    # h*b2 per half: hb2[p, j]
    hb2_d = nc.declare_dram_parameter("hb2", [P, ND2], F32, isOutput=False)
    traj_d = nc.declare_dram_parameter("traj", [n_steps, ND2, P, BC], F32, isOutput=True)

    with TileContext(nc) as tc:
        with (
            tc.tile_pool(name="consts", bufs=1) as cpool,
            tc.tile_pool(name="state", bufs=2) as spool,
            tc.tile_pool(name="xr", bufs=2) as xrpool,
            tc.tile_pool(name="stage", bufs=3) as apool,
            tc.tile_pool(name="th", bufs=3) as thpool,
            tc.tile_pool(name="vtmp", bufs=2) as vpool,
            tc.tile_pool(name="ph", bufs=2, space="PSUM") as phpool,
            tc.tile_pool(name="pq", bufs=3, space="PSUM") as pqpool,
        ):
            def load_const(name, dram, shape, dt):
                t = cpool.tile(shape, dt, tag=name)
                nc.sync.dma_start(out=t[:], in_=dram[:])
                return t

            w1 = load_const("w1", w1_d, [P, ND2, ND2, P], F32R)
            w2h = load_const("w2h", w2h_d, [P, ND2, ND2, P], BF16)
            w2f = load_const("w2f", w2f_d, [P, ND2, ND2, P], BF16)
            w2s = load_const("w2s", w2s_d, [P, ND2, ND2, P], BF16)
            biases = load_const("biases", bias_d, [P, 4, ND2], F32)
            hb2 = load_const("hb2", hb2_d, [P, ND2], F32)

            xs = []
            for st in range(N_STREAMS):
                nb = STREAM_SIZES[st]
                off = STREAM_OFFS[st]
                x = spool.tile([P, ND2, nb], F32, tag=f"x{st}", name=f"x{st}")
                nc.sync.dma_start(
                    out=x[:],
                    in_=x0_d[:, :, off:off + nb].rearrange("j p b -> p j b"),
                )
                xs.append(x)

            def mm_group(out_ps, w_sb, rhs_sb, start=True):
                """out_ps[:, m, :] (+)= sum_k w[:, k, m, :]^T @ rhs[:, k, :]"""
                for m in range(ND2):
                    for k in range(ND2):
                        nc.tensor.matmul(
                            out_ps[:, m, :],
                            w_sb[:, k, m, :],
                            rhs_sb[:, k, :],
                            start=start and (k == 0),
                            stop=(k == ND2 - 1),
                        )

            def vf(st, rhs_sb, w2x, ev):
                """vector-field eval: returns PSUM tile q = c * k' (scaled by w2x)."""
                nb = STREAM_SIZES[st]
                h = phpool.tile([P, ND2, nb], F32, tag="h", name=f"h{st}")
                mm_group(h, w1, rhs_sb)
                th = thpool.tile([P, ND2, nb], BF16, tag=f"th{st}", name=f"th{st}")
                if with_bias:
                    for j in range(ND2):
                        nc.scalar.activation(
                            th[:, j, :], h[:, j, :], TANH, bias=biases[:, ev, j:j+1]
                        )
                elif split_tanh:
                    for j in range(ND2):
                        nc.scalar.activation(th[:, j, :], h[:, j, :], TANH)
                else:
                    nc.scalar.activation(th[:], h[:], TANH)
                q = pqpool.tile([P, ND2, nb], F32, tag=f"q{st}", name=f"q{st}")
                mm_group(q, w2x, th)
                return q

            @contextmanager
            def low_priority(offset=100000):
                old = tc.cur_priority
                tc.cur_priority = old + offset
                try:
                    yield
                finally:
                    tc.cur_priority = old

            # interleave the two streams' step bodies for scheduling slack
            def emit_step(s):
                xcur = [xs[st] for st in range(N_STREAMS)]
                xr = [None] * N_STREAMS
                q1 = [None] * N_STREAMS
                q3 = [None] * N_STREAMS
                q4 = [None] * N_STREAMS
                a2 = [None] * N_STREAMS
                a3 = [None] * N_STREAMS
                a4 = [None] * N_STREAMS

                for st in range(N_STREAMS):
                    # 4-byte rounded copy of state for the first matmul (DVE:
                    # same engine as the xn producer, so no cross-engine hop)
                    xr[st] = xrpool.tile([P, ND2, STREAM_SIZES[st]], F32R, tag=f"xr{st}", name=f"xr{st}")
                    nc.vector.tensor_copy(xr[st][:], xcur[st][:])
                for st in range(N_STREAMS):
                    q1[st] = vf(st, xr[st], w2h, 0)
                for st in range(N_STREAMS):
                    a2[st] = apool.tile([P, ND2, STREAM_SIZES[st]], F32R, tag=f"a{st}", name=f"a2_{st}")
                    nc.vector.tensor_tensor(a2[st][:], xcur[st][:], q1[st][:], ADD)
                q2 = [None] * N_STREAMS
                for st in range(N_STREAMS):
                    q2[st] = vf(st, a2[st], w2h, 1)
                for st in range(N_STREAMS):
                    a3[st] = apool.tile([P, ND2, STREAM_SIZES[st]], F32R, tag=f"a{st}", name=f"a3_{st}")
                    nc.vector.tensor_tensor(a3[st][:], xcur[st][:], q2[st][:], ADD)
                # exact xn accumulation in fp32 directly from the q banks:
                # u1 = x + q1/3 ; u2 = u1 + (2/3) q2 ; v3 = u2 + q3 ; xn = v3 + q4
                u1s = [None] * N_STREAMS
                for st in range(N_STREAMS):
                    with low_priority(400):
                        u1s[st] = vpool.tile(
                            [P, ND2, STREAM_SIZES[st]], F32,
                            tag=f"u1{st}", name=f"u1_{st}"
                        )
                        nc.vector.scalar_tensor_tensor(
                            u1s[st][:], q1[st][:], 1.0 / 3.0, xcur[st][:], MULT, ADD
                        )
                u2s = [None] * N_STREAMS
                for st in range(N_STREAMS):
                    with low_priority(400):
                        u2s[st] = vpool.tile(
                            [P, ND2, STREAM_SIZES[st]], F32,
                            tag=f"u2{st}", name=f"u2_{st}"
                        )
                        nc.vector.scalar_tensor_tensor(
                            u2s[st][:], q2[st][:], 2.0 / 3.0, u1s[st][:], MULT, ADD
                        )
                for st in range(N_STREAMS):
                    q3[st] = vf(st, a3[st], w2f, 2)
                for st in range(N_STREAMS):
                    a4[st] = apool.tile([P, ND2, STREAM_SIZES[st]], F32R, tag=f"a{st}", name=f"a4_{st}")
                    nc.vector.scalar_tensor_tensor(
                        a4[st][:], q3[st][:], 3.0, xcur[st][:], MULT, ADD
                    )
                v3s = [None] * N_STREAMS
                for st in range(N_STREAMS):
                    with low_priority(400):
                        v3s[st] = vpool.tile(
                            [P, ND2, STREAM_SIZES[st]], F32,
                            tag=f"v3{st}", name=f"v3_{st}"
                        )
                        nc.vector.tensor_tensor(
                            v3s[st][:], u2s[st][:], q3[st][:], ADD
                        )
                for st in range(N_STREAMS):
                    q4[st] = vf(st, a4[st], w2s, 3)
                for st in range(N_STREAMS):
                    # xn = (a2 + 2*a3)/3 + q3 + q4 + h*b2
                    v3 = v3s[st]
                    xn = spool.tile([P, ND2, STREAM_SIZES[st]], F32, tag=f"x{st}", name=f"xn{st}")
                    if with_bias:
                        for j in range(ND2):
                            nc.vector.scalar_tensor_tensor(
                                xn[:, j, :], q4[st][:, j, :], hb2[:, j:j+1],
                                v3[:, j, :], ADD, ADD,
                            )
                    else:
                        nc.vector.tensor_tensor(xn[:], v3[:], q4[st][:], ADD)
                    with low_priority():
                        nb = STREAM_SIZES[st]
                        off = STREAM_OFFS[st]
                        nc.sync.dma_start(
                            out=traj_d[s][:, :, off:off + nb].rearrange("j p b -> p j b"),
                            in_=xn[:],
                        )
                    xs[st] = xn

            for s in range(n_steps):
                emit_step(s)

    nc.finalize()
    return nc


_BUILT = {}


def _get_kernel(n_steps=N_STEPS, with_bias=False):
    key = (n_steps, with_bias)
    if key not in _BUILT:
        _BUILT[key] = build_kernel(n_steps, with_bias)
    return _BUILT[key]


def _host_inputs(x0, W1, b1, W2, b2, n_steps=N_STEPS):
    """Build per-core input maps."""
    x0 = np.asarray(x0, dtype=np.float32)
    W1 = np.asarray(W1, dtype=np.float32)
    W2 = np.asarray(W2, dtype=np.float32)
    b1 = np.asarray(b1, dtype=np.float32)
    b2 = np.asarray(b2, dtype=np.float32)

    def wt(w, dt):
        return np.ascontiguousarray(
            w.reshape(ND2, P, ND2, P).transpose(1, 0, 2, 3)
        ).astype(dt)
    import ml_dtypes
    bf = ml_dtypes.bfloat16
    w1t = wt(W1, np.float32)
    w2h = wt(W2 * (DT / 2), bf)
    w2f = wt(W2 * (DT / 3), bf)
    w2s = wt(W2 * (DT / 6), bf)

    b2W1 = b2 @ W1  # (H,)
    evc = [0.0, DT / 2, DT / 2, DT]  # bias shift per eval
    biases = np.stack([b1 + c * b2W1 for c in evc], axis=1)  # (H, 4)
    biases = biases.reshape(ND2, P, 4).transpose(1, 2, 0)     # (P, 4, ND2)
    biases = np.ascontiguousarray(biases).astype(np.float32)
    hb2 = np.ascontiguousarray((DT * b2).reshape(ND2, P).T).astype(np.float32)

    in_maps = []
    for c in range(NCORES):
        shard = x0[c * BC:(c + 1) * BC]                       # (BC, D)
        x0t = np.ascontiguousarray(shard.T).reshape(ND2, P, BC).astype(np.float32)
        in_maps.append({
            "x0t": x0t, "w1t": w1t, "w2h": w2h, "w2f": w2f, "w2s": w2s,
            "biases": biases, "hb2": hb2,
        })
    return in_maps


def run(x0, t_span, W1, b1, W2, b2, n_steps=N_STEPS, trace=False):
    with_bias = bool(
        np.any(np.asarray(b2, dtype=np.float32) != 0.0)
        or np.any(np.asarray(b1, dtype=np.float32) != 0.0)
    )
    nc = _get_kernel(n_steps, with_bias)
    in_maps = _host_inputs(x0, W1, b1, W2, b2, n_steps)
    res = run_bass_kernel_spmd(nc, in_maps, core_ids=list(range(NCORES)), trace=trace)

    t0 = float(np.asarray(t_span, dtype=np.float32)[0])
    times = (t0 + DT * np.arange(n_steps + 1)).astype(np.float32)

    trajectory = np.empty((n_steps + 1, B, D), dtype=np.float32)
    trajectory[0] = np.asarray(x0, dtype=np.float32)
    for c in range(NCORES):
        tc_ = res.results[c]["traj"]                          # (n_steps, 2, 128, BC)
        trajectory[1:, c * BC:(c + 1) * BC, :] = (
            tc_.reshape(n_steps, D, BC).transpose(0, 2, 1)
        )
    return times, trajectory, res


def kernel(x0, t_span, W1, b1, W2, b2):
    times, trajectory, _ = run(x0, t_span, W1, b1, W2, b2)
    return times, trajectory
